# revision 24
# baseline (speedup 1.0000x reference)
"""Trainium2 Bass kernel for nn_CAD_13211319403325 (retrieval_knn).

Reference computation (per sample b of 8, hw=3136 rows, c=1792 dims,
n=2048 centroids):
    d2[row, j] = |x_row|^2 + |c_j|^2 - 2 x_row . c_j          (squared dist)
    dkj = 6 smallest d2 per row (ascending), dk = sqrt of 3 smallest
    score pixel = softmax(-dk)[0] * dk[0]  -> (b, 1, 56, 56)
    loss = soft-boundary terms from dkj (scalar)

Device strategy (data-parallel: one sample per NeuronCore):
    t[row, j] = 2 x_row . c_j - |c_j|^2        = |x_row|^2 - d2[row, j]
    The per-row additive constant |x_row|^2 does not change which j are
    selected, so the 8 *largest* t per row (hardware DVE max8 instruction)
    are exactly the 8 *smallest* d2 per row.  Per 128-row tile:
      - fp32r matmuls (full PE rate) accumulate 2*X @ C into PSUM
      - one DVE tensor_tensor adds -|c|^2 (bias) while draining PSUM->SBUF
      - one DVE max8 gives the top-8 t values, descending
    Device output is just (3136, 8) per core; the tiny epilogue (sqrt,
    softmax, loss reduction, reshape) runs on host in float64.
"""

import os
import numpy as np

# ---- problem constants (hardcoded per contract) ----
B, HW, C, N = 8, 3136, 1792, 2048
K_SEL, J_SEL = 3, 3
NU, ALPHA = 1e-3, 0.1
P = 128          # partitions
KT = C // P      # 14 contraction tiles
M_PAD = 3200     # rows padded to 25*128
MT = M_PAD // P  # 25 row tiles
NB = N // 512    # 4 psum banks per row tile
N_CORES = 8

_PROGRAM_CACHE = {}
LAST_RESULT = None  # BassKernelResults from the most recent run (for test.py)


def _build_program():
    import concourse.bass as bass
    import concourse.tile as tile
    from concourse import bacc, mybir

    f32 = mybir.dt.float32
    f32r = mybir.dt.float32r

    nc = bacc.Bacc(
        "TRN2",
        target_bir_lowering=False,
        debug=False,
        num_devices=N_CORES,
    )

    lhsT_d = nc.dram_tensor("lhsT", [MT, P, KT, P], f32r, kind="ExternalInput")
    cents_d = nc.dram_tensor("cents", [KT, P, N], f32r, kind="ExternalInput")
    negc2_d = nc.dram_tensor("negc2", [P, N], f32, kind="ExternalInput")
    t8_d = nc.dram_tensor("t8", [P, MT * 8], f32, kind="ExternalOutput")

    with tile.TileContext(nc) as tc:
        with (
            tc.tile_pool(name="const", bufs=1) as const_pool,
            tc.tile_pool(name="lhsT", bufs=4) as lhsT_pool,
            tc.tile_pool(name="s", bufs=4) as s_pool,
            tc.tile_pool(name="out", bufs=1) as out_pool,
            tc.tile_pool(name="psum", bufs=2, space="PSUM") as psum_pool,
        ):
            # HAM warm-up: 16 back-to-back matmuls on a zeroed tile with no
            # DMA dependency warm the PE clock gate (1.2 -> 2.4 GHz takes
            # ~3.4us of sustained PE activity) while inputs stream in.  The
            # first real accumulation group (start=True) discards their
            # psum contents.
            H = N // 2  # 1024: half-width accumulation unit (2 psum banks)

            warm_z = const_pool.tile([P, 512], f32, tag="warmz")
            nc.gpsimd.memset(warm_z[:], 0.0)
            warm_t = const_pool.tile([P, 512], f32r, tag="warm")
            nc.vector.tensor_copy(warm_t[:], warm_z[:])
            warm_ps = psum_pool.tile([P, H], f32, tag="psh", bufs=4)
            for i in range(16):
                nc.tensor.matmul(
                    warm_ps[:, 0:512],
                    warm_t[:, 0:P],
                    warm_t[:],
                    start=(i == 0),
                    stop=(i == 15),
                )

            # Centroids stream as 28 half-chunks (columns 0:1024 for every k
            # first, then 1024:2048).  During the stream, row-tiles 0..3 each
            # accumulate into a half-width PSUM tile, so four tiles ride the
            # first-half stream and the PE saturates ~5us in.  Steady-state
            # row-tiles use a pair of half tiles with the same 4-bank
            # rotation as a full-width accumulator.
            cent_ts = [[None, None] for _ in range(KT)]
            lts = {}

            def load_cent(k, h):
                ct = const_pool.tile([P, H], f32r, tag=f"cent{k}h{h}")
                nc.sync.dma_start(ct[:], cents_d[k][:, h * H:(h + 1) * H])
                cent_ts[k][h] = ct

            def load_lhsT(m):
                lt = lhsT_pool.tile([P, KT, P], f32r, tag="lt")
                nc.sync.dma_start(lt[:], lhsT_d[m])
                lts[m] = lt

            load_cent(0, 0)
            load_lhsT(0)
            load_cent(1, 0)
            load_cent(2, 0)
            load_lhsT(1)
            for k in range(3, 8):
                load_cent(k, 0)
            negc2_t = const_pool.tile([P, N], f32)
            nc.sync.dma_start(negc2_t[:], negc2_d[:])
            for k in range(8, KT):
                load_cent(k, 0)
            load_cent(0, 1)
            load_cent(1, 1)
            load_lhsT(2)
            load_cent(2, 1)
            load_cent(3, 1)
            load_lhsT(3)
            for k in range(4, KT):
                load_cent(k, 1)
            out8 = out_pool.tile([P, MT * 8], f32)

            N_STREAM = 4  # row-tiles that ride the centroid stream

            def accum_half(lt, st, t16, h, pst):
                """Accumulate one N-half into pst, drain with bias into st,
                and reduce to the half's top-8 in t16[:, h*8:(h+1)*8].  The
                per-half max8 lets the reduction overlap the other half's
                matmuls; max8 over the 16 candidates then yields the row
                top-8."""
                for k in range(KT):
                    for nb in range(2):
                        nc.tensor.matmul(
                            pst[:, nb * 512:(nb + 1) * 512],
                            lt[:, k, :],
                            cent_ts[k][h][:, nb * 512:(nb + 1) * 512],
                            start=(k == 0),
                            stop=(k == KT - 1),
                        )
                nc.vector.tensor_add(
                    st[:, h * H:(h + 1) * H],
                    pst[:],
                    negc2_t[:, h * H:(h + 1) * H],
                )
                nc.vector.max(
                    t16[:, h * 8:(h + 1) * 8], st[:, h * H:(h + 1) * H]
                )

            # Stream phase: all h0 accumulations for m=0..3, then all h1.
            stream_st = []
            for m in range(N_STREAM):
                lt = lts[m]
                st = s_pool.tile([P, N], f32, tag="st")
                t16 = out_pool.tile([P, 16], f32, tag=f"t16_{m % 2}")
                stream_st.append((st, t16))
                pst = psum_pool.tile([P, H], f32, tag="psh", bufs=4)
                accum_half(lt, st, t16, 0, pst)
            for m in range(N_STREAM):
                st, t16 = stream_st[m]
                pst = psum_pool.tile([P, H], f32, tag="psh", bufs=4)
                accum_half(lts.pop(m), st, t16, 1, pst)
                nc.vector.max(out8[:, m * 8:(m + 1) * 8], t16[:])

            # Steady phase: full-width accumulation via a pair of half
            # tiles, matmuls rotating over all 4 banks per k chunk.
            for m in range(N_STREAM, MT):
                if m not in lts:
                    load_lhsT(m)
                lt = lts.pop(m)
                st = s_pool.tile([P, N], f32, tag="st")
                t16 = out_pool.tile([P, 16], f32, tag=f"t16_{m % 2}")
                pa = psum_pool.tile([P, H], f32, tag="psh", bufs=4)
                pb = psum_pool.tile([P, H], f32, tag="psh", bufs=4)
                for k in range(KT):
                    for h, pst in ((0, pa), (1, pb)):
                        for nb in range(2):
                            nc.tensor.matmul(
                                pst[:, nb * 512:(nb + 1) * 512],
                                lt[:, k, :],
                                cent_ts[k][h][:, nb * 512:(nb + 1) * 512],
                                start=(k == 0),
                                stop=(k == KT - 1),
                            )
                nc.vector.tensor_add(st[:, 0:H], pa[:], negc2_t[:, 0:H])
                nc.vector.max(t16[:, 0:8], st[:, 0:H])
                nc.vector.tensor_add(st[:, H:N], pb[:], negc2_t[:, H:N])
                nc.vector.max(t16[:, 8:16], st[:, H:N])
                nc.vector.max(out8[:, m * 8:(m + 1) * 8], t16[:])

            nc.sync.dma_start(t8_d[:], out8[:])

    nc.compile()
    return nc


def _get_program():
    if "nc" not in _PROGRAM_CACHE:
        _PROGRAM_CACHE["nc"] = _build_program()
    return _PROGRAM_CACHE["nc"]


def kernel(embeds, centroids, r, scale):
    global LAST_RESULT
    from concourse.bass_utils import run_bass_kernel_spmd

    embeds = np.asarray(embeds, dtype=np.float32)
    centroids = np.asarray(centroids, dtype=np.float32)
    r = np.asarray(r, dtype=np.float32)
    scale = int(scale)

    # ---- host-side input prep ----
    # lhsT[core][m, p, k, j] = embeds[core, m*128+j, k*128+p]
    xp = np.zeros((B, M_PAD, C), dtype=np.float32)
    xp[:, :HW] = embeds
    lhsT = np.ascontiguousarray(
        xp.reshape(B, MT, P, KT, P).transpose(0, 1, 4, 3, 2)
    )
    cents = np.ascontiguousarray((2.0 * centroids).reshape(KT, P, N))
    c2 = (centroids.astype(np.float64) ** 2).sum(axis=0)  # |c_j|^2
    negc2 = np.ascontiguousarray(
        np.broadcast_to(-c2.astype(np.float32), (P, N))
    )
    feats = (embeds.astype(np.float64) ** 2).sum(axis=2)  # (B, HW)

    in_maps = [
        {"lhsT": lhsT[core], "cents": cents, "negc2": negc2}
        for core in range(N_CORES)
    ]

    nc = _get_program()
    trace = bool(int(os.environ.get("KERNEL_TRACE", "0")))
    res = None
    for attempt in range(3):
        try:
            res = run_bass_kernel_spmd(nc, in_maps, list(range(N_CORES)),
                                       trace=trace)
            break
        except Exception:
            # Transient NRT_EXEC_UNIT_UNRECOVERABLE wedges clear on retry.
            if attempt == 2:
                raise
    LAST_RESULT = res

    # ---- host-side epilogue (tiny: 8 values per row) ----
    t8 = np.stack([res.results[i]["t8"] for i in range(N_CORES)])  # (B,128,200)
    t8 = (
        t8.reshape(B, P, MT, 8)
        .transpose(0, 2, 1, 3)
        .reshape(B, M_PAD, 8)[:, :HW]
        .astype(np.float64)
    )
    d2_8 = feats[:, :, None] - t8          # 8 smallest d2 per row, ascending
    top3 = d2_8[..., :K_SEL]
    r2 = float(r.astype(np.float64)[0]) ** 2

    # score path
    dk = np.sqrt(np.maximum(top3, 0.0))
    e = np.exp(-(dk - dk.min(axis=-1, keepdims=True)))
    w0 = e[..., 0] / e.sum(axis=-1)
    score = (w0 * dk[..., 0]).reshape(B, scale, HW // scale)[:, None, :, :]

    # loss path (on squared distances, matching reference)
    l_att = (1.0 / NU) * np.mean(np.maximum(top3 - r2, 0.0))
    s_rep = r2 - d2_8[..., K_SEL:K_SEL + J_SEL].mean(axis=-1) + top3.mean(axis=-1)
    l_rep = (1.0 / NU) * np.mean(np.maximum(s_rep - ALPHA, 0.0))
    loss = np.float32(l_att + l_rep)

    return np.asarray(loss, dtype=np.float32), score.astype(np.float32)


# revision 26
# speedup vs baseline: 1.0364x; 1.0364x over previous
"""Trainium2 Bass kernel for nn_CAD_13211319403325 (retrieval_knn).

Reference computation (per sample b of 8, hw=3136 rows, c=1792 dims,
n=2048 centroids):
    d2[row, j] = |x_row|^2 + |c_j|^2 - 2 x_row . c_j          (squared dist)
    dkj = 6 smallest d2 per row (ascending), dk = sqrt of 3 smallest
    score pixel = softmax(-dk)[0] * dk[0]  -> (b, 1, 56, 56)
    loss = soft-boundary terms from dkj (scalar)

Device strategy (data-parallel: one sample per NeuronCore):
    t[row, j] = 2 x_row . c_j - |c_j|^2        = |x_row|^2 - d2[row, j]
    The per-row additive constant |x_row|^2 does not change which j are
    selected, so the 8 *largest* t per row (hardware DVE max8 instruction)
    are exactly the 8 *smallest* d2 per row.  Per 128-row tile:
      - fp32r matmuls (full PE rate) accumulate 2*X @ C into PSUM
      - one DVE tensor_tensor adds -|c|^2 (bias) while draining PSUM->SBUF
      - one DVE max8 gives the top-8 t values, descending
    Device output is just (3136, 8) per core; the tiny epilogue (sqrt,
    softmax, loss reduction, reshape) runs on host in float64.
"""

import os
import numpy as np

# ---- problem constants (hardcoded per contract) ----
B, HW, C, N = 8, 3136, 1792, 2048
K_SEL, J_SEL = 3, 3
NU, ALPHA = 1e-3, 0.1
P = 128          # partitions
KT = C // P      # 14 contraction tiles
M_PAD = 3200     # rows padded to 25*128
MT = M_PAD // P  # 25 row tiles
NB = N // 512    # 4 psum banks per row tile
N_CORES = 8

_PROGRAM_CACHE = {}
LAST_RESULT = None  # BassKernelResults from the most recent run (for test.py)


def _build_program():
    import concourse.bass as bass
    import concourse.tile as tile
    from concourse import bacc, mybir

    f32 = mybir.dt.float32
    f32r = mybir.dt.float32r

    nc = bacc.Bacc(
        "TRN2",
        target_bir_lowering=False,
        debug=False,
        num_devices=N_CORES,
    )

    lhsT_d = nc.dram_tensor("lhsT", [MT, P, KT, P], f32r, kind="ExternalInput")
    cents_d = nc.dram_tensor("cents", [KT, P, N], f32r, kind="ExternalInput")
    negc2_d = nc.dram_tensor("negc2", [P, N], f32, kind="ExternalInput")
    t8_d = nc.dram_tensor("t8", [P, MT * 8], f32, kind="ExternalOutput")

    with tile.TileContext(nc) as tc:
        with (
            tc.tile_pool(name="const", bufs=1) as const_pool,
            tc.tile_pool(name="lhsT", bufs=5) as lhsT_pool,
            tc.tile_pool(name="s", bufs=3) as s_pool,
            tc.tile_pool(name="out", bufs=1) as out_pool,
            tc.tile_pool(name="psum", bufs=2, space="PSUM") as psum_pool,
        ):
            # HAM warm-up: 16 back-to-back matmuls on a zeroed tile with no
            # DMA dependency warm the PE clock gate (1.2 -> 2.4 GHz takes
            # ~3.4us of sustained PE activity) while inputs stream in.  The
            # first real accumulation group (start=True) discards their
            # psum contents.
            H = N // 2  # 1024: half-width accumulation unit (2 psum banks)

            warm_z = const_pool.tile([P, 512], f32, tag="warmz")
            nc.gpsimd.memset(warm_z[:], 0.0)
            warm_t = const_pool.tile([P, 512], f32r, tag="warm")
            nc.vector.tensor_copy(warm_t[:], warm_z[:])
            warm_ps = psum_pool.tile([P, H], f32, tag="psh", bufs=4)
            for i in range(16):
                nc.tensor.matmul(
                    warm_ps[:, 0:512],
                    warm_t[:, 0:P],
                    warm_t[:],
                    start=(i == 0),
                    stop=(i == 15),
                )

            # Centroids stream as 28 half-chunks (columns 0:1024 for every k
            # first, then 1024:2048).  During the stream, row-tiles 0..3 each
            # accumulate into a half-width PSUM tile, so four tiles ride the
            # first-half stream and the PE saturates ~5us in.  Steady-state
            # row-tiles use a pair of half tiles with the same 4-bank
            # rotation as a full-width accumulator.
            cent_ts = [[None, None] for _ in range(KT)]
            lts = {}

            def load_cent(k, h):
                ct = const_pool.tile([P, H], f32r, tag=f"cent{k}h{h}")
                nc.sync.dma_start(ct[:], cents_d[k][:, h * H:(h + 1) * H])
                cent_ts[k][h] = ct

            def load_lhsT(m):
                lt = lhsT_pool.tile([P, KT, P], f32r, tag="lt")
                nc.sync.dma_start(lt[:], lhsT_d[m])
                lts[m] = lt

            # DMA issue order follows first-use time: chunk k of h0 is
            # consumed by stream tile m as soon as it lands; lhsT[m] for the
            # four stream tiles interleave among the early h0 chunks; the
            # first two steady-phase lhsT prefetch during the h1 stream.
            load_cent(0, 0)
            load_lhsT(0)
            load_cent(1, 0)
            load_cent(2, 0)
            load_lhsT(1)
            load_cent(3, 0)
            load_cent(4, 0)
            load_lhsT(2)
            load_cent(5, 0)
            load_cent(6, 0)
            load_lhsT(3)
            for k in range(7, 11):
                load_cent(k, 0)
            negc2_t = const_pool.tile([P, N], f32)
            nc.sync.dma_start(negc2_t[:], negc2_d[:])
            for k in range(11, KT):
                load_cent(k, 0)
            for k in range(0, 4):
                load_cent(k, 1)
            load_lhsT(4)
            for k in range(4, 9):
                load_cent(k, 1)
            load_lhsT(5)
            for k in range(9, KT):
                load_cent(k, 1)
            out8 = out_pool.tile([P, MT * 8], f32)

            N_STREAM = 4  # row-tiles that ride the centroid stream

            def accum_half(lt, st, t16, h, pst):
                """Accumulate one N-half into pst, drain with bias into st,
                and reduce to the half's top-8 in t16[:, h*8:(h+1)*8].  The
                per-half max8 lets the reduction overlap the other half's
                matmuls; max8 over the 16 candidates then yields the row
                top-8."""
                for k in range(KT):
                    for nb in range(2):
                        nc.tensor.matmul(
                            pst[:, nb * 512:(nb + 1) * 512],
                            lt[:, k, :],
                            cent_ts[k][h][:, nb * 512:(nb + 1) * 512],
                            start=(k == 0),
                            stop=(k == KT - 1),
                        )
                nc.vector.tensor_add(
                    st[:, h * H:(h + 1) * H],
                    pst[:],
                    negc2_t[:, h * H:(h + 1) * H],
                )
                nc.vector.max(
                    t16[:, h * 8:(h + 1) * 8], st[:, h * H:(h + 1) * H]
                )

            # Stream phase: all h0 accumulations for m=0..3, then all h1.
            stream_st = []
            for m in range(N_STREAM):
                lt = lts[m]
                st = s_pool.tile([P, N], f32, tag="st")
                t16 = out_pool.tile([P, 16], f32, tag=f"t16_{m % 2}")
                stream_st.append((st, t16))
                pst = psum_pool.tile([P, H], f32, tag="psh", bufs=4)
                accum_half(lt, st, t16, 0, pst)
            for m in range(N_STREAM):
                st, t16 = stream_st[m]
                pst = psum_pool.tile([P, H], f32, tag="psh", bufs=4)
                accum_half(lts.pop(m), st, t16, 1, pst)
                nc.vector.max(out8[:, m * 8:(m + 1) * 8], t16[:])

            # Steady phase: full-width accumulation via a pair of half
            # tiles, matmuls rotating over all 4 banks per k chunk.
            for m in range(N_STREAM, MT):
                if m not in lts:
                    load_lhsT(m)
                lt = lts.pop(m)
                st = s_pool.tile([P, N], f32, tag="st")
                t16 = out_pool.tile([P, 16], f32, tag=f"t16_{m % 2}")
                pa = psum_pool.tile([P, H], f32, tag="psh", bufs=4)
                pb = psum_pool.tile([P, H], f32, tag="psh", bufs=4)
                for k in range(KT):
                    for h, pst in ((0, pa), (1, pb)):
                        for nb in range(2):
                            nc.tensor.matmul(
                                pst[:, nb * 512:(nb + 1) * 512],
                                lt[:, k, :],
                                cent_ts[k][h][:, nb * 512:(nb + 1) * 512],
                                start=(k == 0),
                                stop=(k == KT - 1),
                            )
                nc.vector.tensor_add(st[:, 0:H], pa[:], negc2_t[:, 0:H])
                nc.vector.max(t16[:, 0:8], st[:, 0:H])
                nc.vector.tensor_add(st[:, H:N], pb[:], negc2_t[:, H:N])
                nc.vector.max(t16[:, 8:16], st[:, H:N])
                nc.vector.max(out8[:, m * 8:(m + 1) * 8], t16[:])

            nc.sync.dma_start(t8_d[:], out8[:])

    nc.compile()
    return nc


def _get_program():
    if "nc" not in _PROGRAM_CACHE:
        _PROGRAM_CACHE["nc"] = _build_program()
    return _PROGRAM_CACHE["nc"]


def kernel(embeds, centroids, r, scale):
    global LAST_RESULT
    from concourse.bass_utils import run_bass_kernel_spmd

    embeds = np.asarray(embeds, dtype=np.float32)
    centroids = np.asarray(centroids, dtype=np.float32)
    r = np.asarray(r, dtype=np.float32)
    scale = int(scale)

    # ---- host-side input prep ----
    # lhsT[core][m, p, k, j] = embeds[core, m*128+j, k*128+p]
    xp = np.zeros((B, M_PAD, C), dtype=np.float32)
    xp[:, :HW] = embeds
    lhsT = np.ascontiguousarray(
        xp.reshape(B, MT, P, KT, P).transpose(0, 1, 4, 3, 2)
    )
    cents = np.ascontiguousarray((2.0 * centroids).reshape(KT, P, N))
    c2 = (centroids.astype(np.float64) ** 2).sum(axis=0)  # |c_j|^2
    negc2 = np.ascontiguousarray(
        np.broadcast_to(-c2.astype(np.float32), (P, N))
    )
    feats = (embeds.astype(np.float64) ** 2).sum(axis=2)  # (B, HW)

    in_maps = [
        {"lhsT": lhsT[core], "cents": cents, "negc2": negc2}
        for core in range(N_CORES)
    ]

    nc = _get_program()
    trace = bool(int(os.environ.get("KERNEL_TRACE", "0")))
    res = None
    for attempt in range(3):
        try:
            res = run_bass_kernel_spmd(nc, in_maps, list(range(N_CORES)),
                                       trace=trace)
            break
        except Exception:
            # Transient NRT_EXEC_UNIT_UNRECOVERABLE wedges clear on retry.
            if attempt == 2:
                raise
    LAST_RESULT = res

    # ---- host-side epilogue (tiny: 8 values per row) ----
    t8 = np.stack([res.results[i]["t8"] for i in range(N_CORES)])  # (B,128,200)
    t8 = (
        t8.reshape(B, P, MT, 8)
        .transpose(0, 2, 1, 3)
        .reshape(B, M_PAD, 8)[:, :HW]
        .astype(np.float64)
    )
    d2_8 = feats[:, :, None] - t8          # 8 smallest d2 per row, ascending
    top3 = d2_8[..., :K_SEL]
    r2 = float(r.astype(np.float64)[0]) ** 2

    # score path
    dk = np.sqrt(np.maximum(top3, 0.0))
    e = np.exp(-(dk - dk.min(axis=-1, keepdims=True)))
    w0 = e[..., 0] / e.sum(axis=-1)
    score = (w0 * dk[..., 0]).reshape(B, scale, HW // scale)[:, None, :, :]

    # loss path (on squared distances, matching reference)
    l_att = (1.0 / NU) * np.mean(np.maximum(top3 - r2, 0.0))
    s_rep = r2 - d2_8[..., K_SEL:K_SEL + J_SEL].mean(axis=-1) + top3.mean(axis=-1)
    l_rep = (1.0 / NU) * np.mean(np.maximum(s_rep - ALPHA, 0.0))
    loss = np.float32(l_att + l_rep)

    return np.asarray(loss, dtype=np.float32), score.astype(np.float32)
